# revision 2
# baseline (speedup 1.0000x reference)
"""DeepseekV3 attention (MLA) on 8 TRN2 NeuronCores — v2.

Sharding: phase 1 token-sharded 8 ways (each core owns 256 tokens of EACH
batch); computes kv latents + q latents + full-head q_b for its tokens.
8-way AllToAll exchanges (a) kv latents (batch-duplicated into slots) and
(b) finished q^T (head-group shards), replacing the expensive AllGathers.
Phase 2 head-sharded: core c does batch c//4, heads 4*(c%4)..+3: k_pass/V
from gathered latents, flash-style causal attention, o_proj partial; host
sums the 4 partials per batch (bf16 partials).
"""
import numpy as np
import ml_dtypes

import concourse.bacc as bacc
import concourse.mybir as mybir
import concourse.tile as tile

B, T, HID = 2, 2048, 2048
NH = 16
QLR, KVLR = 1536, 512
DN, DR = 128, 64
DQK, DV = DN + DR, 128
EPS = 1e-6
THETA = 10000.0
SCALE = DQK ** -0.5

NB = 512          # tokens per core in phase 1 (256 from each batch)
HB = 256          # half-block = tokens per batch per core
HPC = 4           # heads per core in phase 2
KVR = KVLR + 2 * DR   # 640 = kv_lat | krot | krotswap

f32 = mybir.dt.float32
bf16 = mybir.dt.bfloat16
Exp = mybir.ActivationFunctionType.Exp
Sqrt = mybir.ActivationFunctionType.Sqrt
Square = mybir.ActivationFunctionType.Square

_BF = ml_dtypes.bfloat16
RG8 = [[0, 1, 2, 3, 4, 5, 6, 7]]


def _build():
    nc = bacc.Bacc(None, num_devices=8)

    xT = nc.declare_dram_parameter("xT", [HID, NB], bf16, isOutput=False)
    wqa = nc.declare_dram_parameter("wqa", [HID, QLR], bf16, isOutput=False)
    wkva = nc.declare_dram_parameter("wkva", [HID, KVR], bf16, isOutput=False)
    wqbr = nc.declare_dram_parameter("wqbr", [QLR, 3072], bf16, isOutput=False)
    sel = nc.declare_dram_parameter("sel", [2, 128, 128], bf16, isOutput=False)
    wkvk = nc.declare_dram_parameter("wkvk", [KVLR, 512], bf16, isOutput=False)
    wkvv = nc.declare_dram_parameter("wkvv", [KVLR, 512], bf16, isOutput=False)
    wo = nc.declare_dram_parameter("wo", [HPC * DV, HID], bf16, isOutput=False)
    cs = nc.declare_dram_parameter("cs", [128, T], bf16, isOutput=False)  # [c;c;-s;s]
    masks = nc.declare_dram_parameter("masks", [4, 128, 1024], bf16, isOutput=False)
    eye2 = nc.declare_dram_parameter("eye2", [128, 128], bf16, isOutput=False)
    out = nc.declare_dram_parameter("out", [T, HID], bf16, isOutput=True)

    a_in_kv = nc.dram_tensor("a_in_kv", [8, KVR, HB], bf16)
    a_out_kv = nc.dram_tensor("a_out_kv", [8, KVR, HB], bf16)
    a_in_q0 = nc.dram_tensor("a_in_q0", [8, 384, HB], bf16)
    a_out_q0 = nc.dram_tensor("a_out_q0", [8, 384, HB], bf16)
    a_in_q1 = nc.dram_tensor("a_in_q1", [8, 384, HB], bf16)
    a_out_q1 = nc.dram_tensor("a_out_q1", [8, 384, HB], bf16)

    with tile.TileContext(nc) as tc:
        # ================= phase 1 =================
        with (
            tc.tile_pool(name="p0", bufs=1) as p0,
            tc.tile_pool(name="ps1", bufs=4, space="PSUM") as ps1,
            tc.tile_pool(name="ps1a", bufs=1, space="PSUM") as ps1a,
        ):
            xt = p0.tile([128, 16, NB], bf16, tag="xt")
            for k in range(16):
                nc.sync.dma_start(xt[:, k], xT[128 * k : 128 * (k + 1), :])
            # wqb chunks allocated up-front (no WAR stall) but their DMA
            # issuance is delayed so the early wire feeds wkva/wqa first
            wqb0 = p0.tile([128, 12, 1536], bf16, tag="wqb0")
            wqb1 = p0.tile([128, 12, 1536], bf16, tag="wqb1")
            ones = p0.tile([128, 128], bf16, tag="ones")
            nc.vector.memset(ones[:], 1.0)
            # raw (unnormalized) q latent; 1/rms folded into the q_b copy
            qlatr = p0.tile([128, 12, NB], bf16, tag="qlatr")
            invq0 = p0.tile([128, NB], f32, tag="invq0")

            # ---- scope A: latents ----
            with (
                tc.tile_pool(name="pa", bufs=1) as pa,
                tc.tile_pool(name="p1w", bufs=2) as p1w,
                tc.tile_pool(name="p1s", bufs=4) as p1s,
            ):
                def rms_inv(ssq, d):
                    # 1/sqrt(mean + eps), [128, NB] f32 (rows identical)
                    mt = p1w.tile([128, NB], f32, tag="rmsm", name="rmsm")
                    nc.vector.tensor_scalar(
                        mt[:], ssq[:], 1.0 / d, EPS,
                        mybir.AluOpType.mult, mybir.AluOpType.add,
                    )
                    rms = p1w.tile([128, NB], f32, tag="rms", name="rms")
                    nc.scalar.activation(rms[:], mt[:], Sqrt)
                    inv = p1w.tile([128, NB], f32, tag="rinv", name="rinv")
                    nc.vector.reciprocal_approx_fast(out=inv[:], in_=rms[:])
                    return inv

                wkvat = pa.tile([128, 16, KVR], bf16, tag="wkvat")
                for k in range(16):
                    nc.gpsimd.dma_start(
                        wkvat[:, k], wkva[128 * k : 128 * (k + 1), :]
                    )
                wqat = pa.tile([128, 16, QLR], bf16, tag="wqat")
                for k in range(16):
                    nc.sync.dma_start(
                        wqat[:, k], wqa[128 * k : 128 * (k + 1), :]
                    )

                # kv latents
                kvlat = pa.tile([128, 4, NB], f32, tag="kvlat")
                ssq_kv = ps1a.tile([128, NB], f32, tag="ssq_kv", name="ssq_kv")
                for m in range(5):
                    ps = ps1.tile([128, NB], f32, tag="p1ps", name="p1ps")
                    for k in range(16):
                        nc.tensor.matmul(
                            ps[:], wkvat[:, k, 128 * m : 128 * (m + 1)],
                            xt[:, k], start=(k == 0), stop=(k == 15),
                        )
                    if m < 4:
                        sq = p1w.tile([128, NB], bf16, tag="sq", name="sq")
                        nc.scalar.activation(sq[:], ps[:], Square)
                        nc.vector.tensor_copy(kvlat[:, m], ps[:])
                        nc.tensor.matmul(ssq_kv[:], ones[:], sq[:],
                                         start=(m == 0), stop=(m == 3))
                    else:
                        rot = p1s.tile([128, NB], bf16, tag="rot", name="rot")
                        nc.vector.tensor_copy(rot[:], ps[:])
                        for j in range(8):
                            h0 = HB * (j // 4)
                            eng = (nc.gpsimd, nc.scalar, nc.sync)[j % 3]
                            eng.dma_start(
                                a_in_kv[j, KVLR:KVR, :], rot[:, h0 : h0 + HB]
                            )
                inv_kv = rms_inv(ssq_kv, KVLR)
                for m in range(4):
                    ltn = p1s.tile([128, NB], bf16, tag="ltn", name="ltn")
                    nc.vector.tensor_mul(ltn[:], kvlat[:, m], inv_kv[:])
                    for j in range(8):
                        h0 = HB * (j // 4)
                        eng = (nc.gpsimd, nc.scalar, nc.sync)[j % 3]
                        eng.dma_start(
                            a_in_kv[j, 128 * m : 128 * (m + 1), :],
                            ltn[:, h0 : h0 + HB],
                        )
                nc.gpsimd.collective_compute(
                    "AllToAll", mybir.AluOpType.bypass, replica_groups=RG8,
                    ins=[a_in_kv[:]], outs=[a_out_kv[:]],
                )

                for k in range(12):
                    nc.scalar.dma_start(
                        wqb0[:, k], wqbr[128 * k : 128 * (k + 1), 0:1536]
                    )

                # q latents (stored raw; norm factored into q_b output)
                ssq_q = ps1a.tile([128, NB], f32, tag="ssq_q", name="ssq_q")
                for m in range(12):
                    ps = ps1.tile([128, NB], f32, tag="p1ps", name="p1psq")
                    for k in range(16):
                        nc.tensor.matmul(
                            ps[:], wqat[:, k, 128 * m : 128 * (m + 1)],
                            xt[:, k], start=(k == 0), stop=(k == 15),
                        )
                    sq = p1w.tile([128, NB], bf16, tag="sqq", name="sqq")
                    nc.scalar.activation(sq[:], ps[:], Square)
                    nc.vector.tensor_copy(qlatr[:, m], ps[:])
                    nc.tensor.matmul(ssq_q[:], ones[:], sq[:],
                                     start=(m == 0), stop=(m == 11))
                for k in range(12):
                    nc.scalar.dma_start(
                        wqb1[:, k], wqbr[128 * k : 128 * (k + 1), 1536:3072]
                    )
                mtq = p1w.tile([128, NB], f32, tag="rmsm", name="rmsmq")
                nc.vector.tensor_scalar(
                    mtq[:], ssq_q[:], 1.0 / QLR, EPS,
                    mybir.AluOpType.mult, mybir.AluOpType.add,
                )
                rmsq = p1w.tile([128, NB], f32, tag="rms", name="rmsq")
                nc.scalar.activation(rmsq[:], mtq[:], Sqrt)
                nc.vector.reciprocal_approx_fast(out=invq0[:], in_=rmsq[:])

            # ---- scope B: q_b (all 16 heads), 2 A2A chunks ----
            with tc.tile_pool(name="p1s2", bufs=4) as p1s2:
                for half, (agi, ago) in enumerate(
                    ((a_in_q0, a_out_q0), (a_in_q1, a_out_q1))
                ):
                    wsrc = wqb0 if half == 0 else wqb1
                    for j in range(12):
                        g, m3 = j // 3, j % 3
                        ps = ps1.tile([128, NB], f32, tag="p1ps", name="qbps")
                        for k in range(12):
                            nc.tensor.matmul(
                                ps[:], wsrc[:, k, 128 * j : 128 * (j + 1)],
                                qlatr[:, k], start=(k == 0), stop=(k == 11),
                            )
                        qbt = p1s2.tile([128, NB], bf16, tag="qbt", name="qbt")
                        nc.vector.tensor_mul(qbt[:], ps[:], invq0[:])
                        nc.gpsimd.dma_start(
                            agi[g, 128 * m3 : 128 * (m3 + 1), :],
                            qbt[:, 0:HB],
                        )
                        nc.scalar.dma_start(
                            agi[4 + g, 128 * m3 : 128 * (m3 + 1), :],
                            qbt[:, HB:NB],
                        )
                    nc.gpsimd.collective_compute(
                        "AllToAll", mybir.AluOpType.bypass, replica_groups=RG8,
                        ins=[agi[:]], outs=[ago[:]],
                    )

        # ================= phase 2 =================
        with tc.tile_pool(name="p2", bufs=1) as p2:
            cst = p2.tile([128, T], bf16, tag="cst")
            nc.sync.dma_start(cst[:], cs[:])
            selt = p2.tile([128, 2, 128], bf16, tag="selt")
            for v in range(2):
                nc.sync.dma_start(selt[:, v], sel[v])
            maskt = p2.tile([128, 4, 2, 512], bf16, tag="maskt")
            for m in range(4):
                nc.sync.dma_start(maskt[:, m], masks[m])
            onesb = p2.tile([128, 128], bf16, tag="onesb")
            nc.vector.memset(onesb[:], 1.0)
            eyed = p2.tile([128, 128], bf16, tag="eyed")
            nc.sync.dma_start(eyed[:], eye2[:])
            wot = p2.tile([128, 4, HID], bf16, tag="wot")
            for k in range(4):
                nc.scalar.dma_start(wot[:, k], wo[128 * k : 128 * (k + 1), :])

            kpT = p2.tile([128, 4, 4, NB], bf16, tag="kpT")   # [d, l, r, t]
            vT = p2.tile([128, 16, 512], bf16, tag="vT")      # [t, t-tile, dv]
            krot2 = p2.tile([128, T], bf16, tag="krot2")
            qTp = p2.tile([128, 4, 4, NB], bf16, tag="qTp")   # [d, l, qn, t]
            qrw = p2.tile([128, 2, T], bf16, tag="qrw")       # raw rot pairs
            qrot2 = p2.tile([128, 2, 4, NB], bf16, tag="qrot2")  # [2d,hp,qn,t]

            # ---- 2b: k_pass, V, k_rot + q rope, A2A-overlap-ordered ----
            with (
                tc.tile_pool(name="p2b", bufs=1) as p2b,
                tc.tile_pool(name="p2bw", bufs=3) as p2bw,
                tc.tile_pool(name="ps2b", bufs=4, space="PSUM") as ps2b,
                tc.tile_pool(name="ps2k", bufs=2, space="PSUM") as ps2k,
            ):
                wkkt = p2b.tile([128, 4, 512], bf16, tag="wkkt")
                wkvt = p2b.tile([128, 4, 512], bf16, tag="wkvt")
                for k in range(4):
                    nc.sync.dma_start(wkkt[:, k], wkvk[128 * k : 128 * (k + 1), :])
                    nc.sync.dma_start(wkvt[:, k], wkvv[128 * k : 128 * (k + 1), :])
                kvl = p2b.tile([128, 4, T], bf16, tag="kvl")  # [r_lat, k, t]
                krr = p2b.tile([128, T], bf16, tag="krr")
                for i in range(8):
                    c0 = HB * i
                    for k in range(4):
                        nc.sync.dma_start(
                            kvl[:, k, c0 : c0 + HB],
                            a_out_kv[i, 128 * k : 128 * (k + 1), :],
                        )
                    nc.sync.dma_start(
                        krr[:, c0 : c0 + HB], a_out_kv[i, KVLR:KVR, :]
                    )
                for half, ago in ((0, a_out_q0), (1, a_out_q1)):
                    for i in range(8):
                        qn, bh = i // 2, i % 2
                        c0 = HB * bh
                        nc.sync.dma_start(
                            qrw[:, half, HB * i : HB * (i + 1)],
                            ago[i, 256:384, :],
                        )
                        for v in range(2):
                            nc.sync.dma_start(
                                qTp[:, 2 * half + v, qn, c0 : c0 + HB],
                                ago[i, 128 * v : 128 * (v + 1), :],
                            )

                def rope_q(half):
                    for qn in range(4):
                        pr = ps2k.tile([128, NB], f32, tag="kropeps",
                                       name="qropeps")
                        for v in range(2):
                            sp = ps2k.tile([128, NB], f32, tag="selps",
                                           name="selps")
                            nc.tensor.matmul(
                                sp[:], selt[:, v],
                                qrw[:, half, NB * qn : NB * (qn + 1)],
                                start=True, stop=True,
                            )
                            tt = p2bw.tile([128, NB], bf16, tag="qropet",
                                           name="qropet")
                            nc.vector.tensor_mul(
                                tt[:], sp[:], cst[:, NB * qn : NB * (qn + 1)]
                            )
                            nc.tensor.matmul(pr[64 * v : 64 * (v + 1), :],
                                             eyed[:, 0:64], tt[:],
                                             start=True, stop=True)
                        nc.vector.tensor_copy(qrot2[:, half, qn], pr[:])

                # k_pass^T per head l per 512-token block r
                for r in range(4):
                    for l in range(4):
                        ps = ps2b.tile([128, NB], f32, tag="k2ps", name="k2ps")
                        for k in range(4):
                            nc.tensor.matmul(
                                ps[:], wkkt[:, k, 128 * l : 128 * (l + 1)],
                                kvl[:, k, NB * r : NB * (r + 1)],
                                start=(k == 0), stop=(k == 3),
                            )
                        if (l + r) % 2 == 0:
                            nc.vector.tensor_copy(kpT[:, l, r], ps[:])
                        else:
                            nc.scalar.copy(kpT[:, l, r], ps[:])
                # V token-major
                for r in range(4):
                    for s in range(4):
                        ps = ps2b.tile([128, 512], f32, tag="k2ps", name="v2ps")
                        for k in range(4):
                            nc.tensor.matmul(
                                ps[:],
                                kvl[:, k, NB * r + 128 * s : NB * r + 128 * (s + 1)],
                                wkvt[:, k], start=(k == 0), stop=(k == 3),
                            )
                        if (r + s) % 2 == 0:
                            nc.vector.tensor_copy(vT[:, 4 * r + s], ps[:])
                        else:
                            nc.scalar.copy(vT[:, 4 * r + s], ps[:])
                # k_rot rope: mul by cs then fold halves via eye-matmul
                for r in range(4):
                    tt = p2bw.tile([128, NB], bf16, tag="kropet", name="kropet")
                    nc.vector.tensor_mul(
                        tt[:], krr[:, NB * r : NB * (r + 1)],
                        cst[:, NB * r : NB * (r + 1)],
                    )
                    pr = ps2k.tile([128, NB], f32, tag="kropeps", name="kropeps")
                    nc.tensor.matmul(pr[:], eyed[:], tt[:], start=True, stop=True)
                    nc.vector.tensor_copy(
                        krot2[:, NB * r : NB * (r + 1)], pr[:]
                    )
                rope_q(0)
                rope_q(1)

            # ---- 2d attention + 2e o_proj interleaved per qn ----
            with (
                tc.tile_pool(name="p2d", bufs=4) as p2d,
                tc.tile_pool(name="p2dn", bufs=2) as p2dn,
                tc.tile_pool(name="p2e", bufs=4) as p2e,
                tc.tile_pool(name="psc", bufs=2, space="PSUM") as psc,
                tc.tile_pool(name="psa", bufs=2, space="PSUM") as psa,
                tc.tile_pool(name="pso", bufs=2, space="PSUM") as pso,
            ):
                for qn in range(4):
                    attnT = p2dn.tile([128, 4, NB], bf16, tag="attnT",
                                      name="attnT")
                    for hp in range(2):
                        l0 = 2 * hp
                        nkt = 4 * qn + 4
                        aps = [
                            psa.tile([128, NB], f32, tag="aps", name="aps")
                            for _ in range(2)
                        ]
                        eac = p2d.tile([128, 2, NB], bf16, tag="eacc",
                                       name="eacc")
                        pend = []

                        def emit_pv(pe, pk, off, last, aps=aps, l0=l0,
                                    eac=eac, qn=qn):
                            for v in range(2):
                                nc.tensor.matmul(
                                    aps[v][:, off:NB],
                                    vT[:, pk, 128 * (l0 + v) : 128 * (l0 + v + 1)],
                                    pe[:, v, off:NB],
                                    start=(pk == 0), stop=last,
                                )
                            if pk == 0:
                                nc.vector.tensor_copy(eac[:], pe[:])
                            else:
                                nc.vector.tensor_add(
                                    eac[:, :, off:NB], eac[:, :, off:NB],
                                    pe[:, :, off:NB],
                                )

                        for kt in range(nkt):
                            m = kt - 4 * qn
                            off = 128 * m if m > 0 else 0
                            scp = psc.tile([128, 2, NB], f32, tag="scp",
                                           name="scp")
                            for v in range(2):
                                nc.tensor.matmul(
                                    scp[:, v, off:NB],
                                    kpT[:, l0 + v, kt // 4,
                                        128 * (kt % 4) : 128 * (kt % 4) + 128],
                                    qTp[:, l0 + v, qn, off:NB],
                                    start=True, stop=False,
                                )
                            # the two 64-contract rot matmuls sit on disjoint
                            # PE row strips -> issued back-to-back they pack
                            for v in range(2):
                                nc.tensor.matmul(
                                    scp[:, v, off:NB],
                                    krot2[64 * v : 64 * (v + 1),
                                          128 * kt : 128 * (kt + 1)],
                                    qrot2[64 * v : 64 * (v + 1), hp, qn, off:NB],
                                    start=False, stop=True,
                                )
                            et = p2d.tile([128, 2, NB], bf16, tag="expT",
                                          name="expT")
                            nc.scalar.activation(
                                et[:, :, off:NB], scp[:, :, off:NB], Exp,
                                scale=SCALE,
                            )
                            if m >= 0:
                                nc.vector.tensor_mul(
                                    et[:, :, off:NB], et[:, :, off:NB],
                                    maskt[:, m, :, off:NB],
                                )
                            if len(pend) == 2:
                                pv = pend.pop(0)
                                emit_pv(pv[0], pv[1], pv[2], False)
                            pend.append((et, kt, off))
                        for pv in pend[:-1]:
                            emit_pv(pv[0], pv[1], pv[2], False)
                        pv = pend[-1]
                        emit_pv(pv[0], pv[1], pv[2], True)
                        sps = psc.tile([128, 2, NB], f32, tag="scp",
                                       name="sps")
                        for v in range(2):
                            nc.tensor.matmul(sps[:, v], onesb[:], eac[:, v],
                                             start=True, stop=True)
                        rec = p2d.tile([128, 2, NB], f32, tag="rec", name="rec")
                        nc.vector.reciprocal_approx_fast(
                            out=rec[:], in_=sps[:]
                        )
                        for v in range(2):
                            nc.vector.tensor_mul(attnT[:, l0 + v], aps[v][:],
                                                 rec[:, v])
                    # 2e for this qn
                    for tm in range(4):
                        for n in range(4):
                            ps = pso.tile([128, 512], f32, tag="ops",
                                          name="ops")
                            for k in range(4):
                                nc.tensor.matmul(
                                    ps[:], attnT[:, k, 128 * tm : 128 * (tm + 1)],
                                    wot[:, k, 512 * n : 512 * (n + 1)],
                                    start=(k == 0), stop=(k == 3),
                                )
                            ot = p2e.tile([128, 512], bf16, tag="oT", name="oT")
                            if (tm + n) % 2 == 0:
                                nc.scalar.copy(ot[:], ps[:])
                            else:
                                nc.vector.tensor_copy(ot[:], ps[:])
                            nc.sync.dma_start(
                                out[NB * qn + 128 * tm : NB * qn + 128 * (tm + 1),
                                    512 * n : 512 * (n + 1)],
                                ot[:],
                            )

    nc.finalize()
    return nc


_NC = None


def _get_nc():
    global _NC
    if _NC is None:
        _NC = _build()
    return _NC


def _prep_inputs(x, attention_mask, positions, wqa, qa_scale, wqb, wkva,
                 kva_scale, wkvb, wo):
    x = np.asarray(x, np.float32)
    positions = np.asarray(positions)
    wqa_ = np.asarray(wqa, np.float32)
    wqb_ = np.asarray(wqb, np.float32) * np.asarray(qa_scale, np.float32)[:, None]
    wkva_ = np.asarray(wkva, np.float32)
    wkvb_ = np.asarray(wkvb, np.float32) * np.asarray(kva_scale, np.float32)[:, None]
    wo_ = np.asarray(wo, np.float32)

    # wkva augmented with swapped-rot columns: [lat | rot | rotswap]
    kr = wkva_[:, KVLR:]
    wkva_aug = np.concatenate(
        [wkva_[:, :KVLR], kr, kr[:, DR // 2 :], kr[:, : DR // 2]], axis=1
    ).astype(_BF)

    # wqb reordered chunk-major: chunk c (1536 cols) = for g: [ha-pass,
    # hb-pass, ha-rot|hb-rot] with (ha, hb) = (4g+2c, 4g+2c+1)
    cols = []
    for c in range(2):
        for g in range(4):
            ha, hb = 4 * g + 2 * c, 4 * g + 2 * c + 1
            cols.append(wqb_[:, ha * DQK : ha * DQK + DN])
            cols.append(wqb_[:, hb * DQK : hb * DQK + DN])
            cols.append(wqb_[:, ha * DQK + DN : (ha + 1) * DQK])
            cols.append(wqb_[:, hb * DQK + DN : (hb + 1) * DQK])
    wqbr = np.concatenate(cols, axis=1).astype(_BF)

    # masks2: mask[m][r, c] = c >= 128*m + r, duplicated for head pairs
    rr = np.arange(128)[:, None]
    cc = np.arange(512)[None, :]
    mk = np.stack([(cc >= 128 * m + rr) for m in range(4)]).astype(_BF)
    masks2 = np.concatenate([mk, mk], axis=2)  # [4, 128, 1024]

    eye2 = np.tile(np.concatenate([np.eye(64), np.eye(64)], axis=0),
                   (1, 2)).astype(_BF)

    # sel[v]: rows[0:64]=src rows[64v:64v+64]; rows[64:128]=32-swapped copy
    sel = np.zeros((2, 128, 128), np.float32)
    for v in range(2):
        for i in range(64):
            sel[v, 64 * v + i, i] = 1.0
            sel[v, 64 * v + ((i + 32) % 64), 64 + i] = 1.0
    sel = sel.astype(_BF)

    # per-batch cos/sin stack [c; c; -s; s]
    inv_freq = 1.0 / (THETA ** (np.arange(0, DR, 2, dtype=np.float32) / DR))
    cs_b = []
    for b in range(B):
        ang = positions[b].astype(np.float32)[None, :] * inv_freq[:, None]
        c, s = np.cos(ang), np.sin(ang)
        cs_b.append(np.concatenate([c, c, -s, s], axis=0).astype(_BF))

    wqa_bf = wqa_.astype(_BF)
    in_maps = []
    for core in range(8):
        b, g = core // 4, core % 4
        hs = [4 * g + i for i in range(HPC)]
        wkvk_hg = np.concatenate(
            [wkvb_[:, h * (DN + DV) : h * (DN + DV) + DN] for h in hs], axis=1
        ).astype(_BF)
        wkvv_hg = np.concatenate(
            [wkvb_[:, h * (DN + DV) + DN : (h + 1) * (DN + DV)] for h in hs],
            axis=1,
        ).astype(_BF)
        wo_hg = wo_[hs[0] * DV : (hs[-1] + 1) * DV, :].astype(_BF)
        xTb = np.ascontiguousarray(
            np.concatenate(
                [x[0, HB * core : HB * (core + 1), :],
                 x[1, HB * core : HB * (core + 1), :]], axis=0
            ).T
        ).astype(_BF)
        in_maps.append({
            "xT": xTb,
            "wqa": wqa_bf,
            "wkva": wkva_aug,
            "wqbr": wqbr,
            "wkvk": wkvk_hg,
            "wkvv": wkvv_hg,
            "wo": wo_hg,
            "cs": cs_b[b],
            "masks": masks2,
            "sel": sel,
            "eye2": eye2,
        })
    return in_maps


def _run(inputs, trace=False, trace_kwargs=None):
    from concourse.bass_utils import run_bass_kernel_spmd

    nc = _get_nc()
    in_maps = _prep_inputs(**inputs)
    res = run_bass_kernel_spmd(
        nc, in_maps, list(range(8)), trace=trace,
        trace_kwargs=trace_kwargs or {},
    )
    outs = np.zeros((B, T, HID), np.float32)
    for core in range(8):
        outs[core // 4] += np.asarray(res.results[core]["out"], np.float32)
    return outs, res


def kernel(**inputs) -> np.ndarray:
    out, _ = _run(inputs)
    return out
